# revision 6
# baseline (speedup 1.0000x reference)
"""DeepCoevolve forward pass as a Bass/Tile kernel for 8 Trainium2 NeuronCores.

Strategy: data-parallel over the event axis (512 events/core), embedding
tables replicated.  Per core, all 4096 paths are processed with a
feature-major layout (H=128 on SBUF partitions, batch on the free axis):

  1. indirect-DMA gathers of path/event embedding rows (row-major tiles)
  2. PE transposes -> feature-major x_t [128, 4096]
  3. 4-step LSTM: fp32r matmuls into PSUM (gate = Wih.T @ x + Whh.T @ h),
     sigmoid/tanh on the ACT engine (bias fused), c/h updates on DVE
  4. GRU cells for user/item (same structure, 512 events in one chunk)
  5. max-pool attention: PE transpose h -> row-major, one wide DVE
     reduce_max, exp on ACT, per-tile softmax-weighted pooling via small
     matmuls against constant pooling matrices
  6. fused linears (PSUM-accumulated matmuls), dot product via ones-vector
     matmul, numerically-stable softplus on ACT
"""
import sys

sys.path.insert(0, "/opt/trn_rl_repo")

import numpy as np

E, P, H = 4096, 8, 128
NU, NI = 100000, 50000
M = 8              # cores
EL = E // M        # events per core
N = EL * P         # paths per core
NT = N // 128      # path tiles per core
ET = EL // 128     # event tiles per core
CH = 512           # batch chunk (PSUM bank free size in fp32)
NCH = N // CH

_CACHE = {}
LAST_RESULT = None


def _build_program():
    import concourse.bass as bass
    import concourse.tile as tile
    from concourse import bacc, mybir

    f32 = mybir.dt.float32
    f32r = mybir.dt.float32r
    i32 = mybir.dt.int32
    AF = mybir.ActivationFunctionType
    OP = mybir.AluOpType

    nc = bacc.Bacc("TRN2", target_bir_lowering=False)

    U = nc.dram_tensor("user_emb", [NU, H], f32, kind="ExternalInput")
    I = nc.dram_tensor("item_emb", [NI, H], f32, kind="ExternalInput")
    GIDX = nc.dram_tensor("gidx", [128, 136], i32, kind="ExternalInput")
    WIH_T = nc.dram_tensor("wih_t", [128, 512], f32, kind="ExternalInput")
    WHH_T = nc.dram_tensor("whh_t", [128, 512], f32, kind="ExternalInput")
    LSTM_B = nc.dram_tensor("lstm_b", [128, 4], f32, kind="ExternalInput")
    GUW_IH = nc.dram_tensor("guw_ih_t", [128, 384], f32, kind="ExternalInput")
    GUW_HH = nc.dram_tensor("guw_hh_t", [128, 384], f32, kind="ExternalInput")
    GIW_IH = nc.dram_tensor("giw_ih_t", [128, 384], f32, kind="ExternalInput")
    GIW_HH = nc.dram_tensor("giw_hh_t", [128, 384], f32, kind="ExternalInput")
    GU_BSUM = nc.dram_tensor("gu_bsum", [128, 3], f32, kind="ExternalInput")
    GU_BI = nc.dram_tensor("gu_bi", [128, 3], f32, kind="ExternalInput")
    GU_BH = nc.dram_tensor("gu_bh", [128, 3], f32, kind="ExternalInput")
    GI_BSUM = nc.dram_tensor("gi_bsum", [128, 3], f32, kind="ExternalInput")
    GI_BI = nc.dram_tensor("gi_bi", [128, 3], f32, kind="ExternalInput")
    GI_BH = nc.dram_tensor("gi_bh", [128, 3], f32, kind="ExternalInput")
    ULW_T = nc.dram_tensor("ulw_t", [256, 128], f32, kind="ExternalInput")
    ILW_T = nc.dram_tensor("ilw_t", [256, 128], f32, kind="ExternalInput")
    ULB = nc.dram_tensor("ulb", [128, 1], f32, kind="ExternalInput")
    ILB = nc.dram_tensor("ilb", [128, 1], f32, kind="ExternalInput")
    POOL_E = nc.dram_tensor("pool_e", [128, 16], f32, kind="ExternalInput")
    POOL_BLK = nc.dram_tensor("pool_blk", [128, 128], f32, kind="ExternalInput")
    IDENT = nc.dram_tensor("ident", [128, 128], f32, kind="ExternalInput")
    ONES = nc.dram_tensor("ones", [128, 1], f32, kind="ExternalInput")
    OUT = nc.dram_tensor("comp", [1, EL], f32, kind="ExternalOutput")

    def r(ap):
        return ap.bitcast(f32r)

    with tile.TileContext(nc) as tc:
        with (
            tc.tile_pool(name="persist", bufs=1) as pp,
            tc.tile_pool(name="gather", bufs=4) as gp,
            tc.tile_pool(name="work", bufs=10) as wp,
            tc.tile_pool(name="pgates", bufs=6, space="PSUM") as pg,
            tc.tile_pool(name="ptr", bufs=2, space="PSUM") as pt,
        ):
            # ---- persistent SBUF state ----
            idx = pp.tile([128, 136], i32, tag="idx")
            wih = pp.tile([128, 512], f32, tag="wih")
            whh = pp.tile([128, 512], f32, tag="whh")
            lb = pp.tile([128, 4], f32, tag="lb")
            guwi = pp.tile([128, 384], f32, tag="guwi")
            guwh = pp.tile([128, 384], f32, tag="guwh")
            giwi = pp.tile([128, 384], f32, tag="giwi")
            giwh = pp.tile([128, 384], f32, tag="giwh")
            gubs = pp.tile([128, 3], f32, tag="gubs")
            gubi = pp.tile([128, 3], f32, tag="gubi")
            gubh = pp.tile([128, 3], f32, tag="gubh")
            gibs = pp.tile([128, 3], f32, tag="gibs")
            gibi = pp.tile([128, 3], f32, tag="gibi")
            gibh = pp.tile([128, 3], f32, tag="gibh")
            ulw0 = pp.tile([128, 128], f32, tag="ulw0")
            ulw1 = pp.tile([128, 128], f32, tag="ulw1")
            ilw0 = pp.tile([128, 128], f32, tag="ilw0")
            ilw1 = pp.tile([128, 128], f32, tag="ilw1")
            ulb = pp.tile([128, 1], f32, tag="ulb")
            ilb = pp.tile([128, 1], f32, tag="ilb")
            poole = pp.tile([128, 16], f32, tag="poole")
            poolb = pp.tile([128, 128], f32, tag="poolb")
            ident = pp.tile([128, 128], f32, tag="ident")
            ones = pp.tile([128, 1], f32, tag="ones")

            xs = [pp.tile([128, N], f32, tag=f"x{t}", name=f"x{t}") for t in range(4)]
            hst = pp.tile([128, N], f32, tag="h")
            cst = pp.tile([128, N], f32, tag="c")
            hrm = pp.tile([128, N], f32, tag="hrm")
            ufm = pp.tile([128, EL], f32, tag="ufm")
            ifm = pp.tile([128, EL], f32, tag="ifm")
            pefm = pp.tile([128, EL], f32, tag="pefm")
            unew = pp.tile([128, EL], f32, tag="unew")
            inew = pp.tile([128, EL], f32, tag="inew")
            scores = pp.tile([128, NT], f32, tag="scores")
            es = pp.tile([128, NT], f32, tag="es")
            compsb = pp.tile([1, EL], f32, tag="compsb")

            nc.sync.dma_start(idx[:], GIDX[:])
            nc.sync.dma_start(lb[:], LSTM_B[:])
            nc.sync.dma_start(gubs[:], GU_BSUM[:])
            nc.sync.dma_start(gubi[:], GU_BI[:])
            nc.sync.dma_start(gubh[:], GU_BH[:])
            nc.sync.dma_start(gibs[:], GI_BSUM[:])
            nc.sync.dma_start(gibi[:], GI_BI[:])
            nc.sync.dma_start(gibh[:], GI_BH[:])
            nc.sync.dma_start(ulb[:], ULB[:])
            nc.sync.dma_start(ilb[:], ILB[:])
            nc.sync.dma_start(poole[:], POOL_E[:])
            nc.sync.dma_start(poolb[:], POOL_BLK[:])
            nc.sync.dma_start(ident[:], IDENT[:])

            # weights consumed by f32r matmuls: DMA raw fp32, round via DVE copy
            def load_r(dst, src_ap):
                raw = gp.tile(list(dst.shape), f32, tag="wraw", name="wraw")
                nc.sync.dma_start(raw[:], src_ap)
                nc.vector.tensor_copy(out=dst[:].bitcast(f32r), in_=raw[:])

            load_r(wih, WIH_T[:])
            load_r(whh, WHH_T[:])
            load_r(guwi, GUW_IH[:])
            load_r(guwh, GUW_HH[:])
            load_r(giwi, GIW_IH[:])
            load_r(giwh, GIW_HH[:])
            load_r(ulw0, ULW_T[0:128, :])
            load_r(ulw1, ULW_T[128:256, :])
            load_r(ilw0, ILW_T[0:128, :])
            load_r(ilw1, ILW_T[128:256, :])
            load_r(ones, ONES[:])

            def gather_t(dst, table, col, to_r=False):
                """Gather 128 rows by idx[:, col] and transpose into
                dst (feature-major [128, 128] column block)."""
                g_rm = gp.tile([128, H], f32, tag="grm")
                nc.gpsimd.indirect_dma_start(
                    out=g_rm[:],
                    out_offset=None,
                    in_=table[:],
                    in_offset=bass.IndirectOffsetOnAxis(ap=idx[:, col:col + 1], axis=0),
                )
                ps = pt.tile([128, 128], f32, tag="ptr")
                nc.tensor.transpose(out=ps[:], in_=g_rm[:], identity=ident[:])
                nc.vector.tensor_copy(out=dst.bitcast(f32r) if to_r else dst,
                                      in_=ps[:])

            # ---- phase 1: gathers ----
            for t in range(4):
                tbl = U if t % 2 == 0 else I
                for j in range(NT):
                    gather_t(xs[t][:, j * 128:(j + 1) * 128], tbl, t * 32 + j,
                             to_r=True)
            for j in range(ET):
                gather_t(ufm[:, j * 128:(j + 1) * 128], U, 128 + j, to_r=True)
                gather_t(ifm[:, j * 128:(j + 1) * 128], I, 132 + j, to_r=True)

            # ---- phase 2: LSTM (gate order i,f,g,o = 0,1,2,3) ----
            for t in range(4):
                for k in range(NCH):
                    ck = slice(k * CH, (k + 1) * CH)

                    def gate_psum(g):
                        ps = pg.tile([128, CH], f32, tag="g")
                        nc.tensor.matmul(
                            out=ps[:], lhsT=r(wih[:, g * 128:(g + 1) * 128]),
                            rhs=r(xs[t][:, ck]), start=True, stop=(t == 0),
                        )
                        if t > 0:
                            nc.tensor.matmul(
                                out=ps[:], lhsT=r(whh[:, g * 128:(g + 1) * 128]),
                                rhs=r(hst[:, ck]), start=False, stop=True,
                            )
                        return ps

                    def act_gate(g, fn):
                        ps = gate_psum(g)
                        o = wp.tile([128, CH], f32, tag="w")
                        nc.scalar.activation(out=o[:], in_=ps[:], func=fn,
                                             bias=lb[:, g:g + 1])
                        return o

                    si = act_gate(0, AF.Sigmoid)
                    tg = act_gate(2, AF.Tanh)
                    so = act_gate(3, AF.Sigmoid)
                    if t == 0:
                        nc.vector.tensor_tensor(out=cst[:, ck], in0=si[:], in1=tg[:],
                                                op=OP.mult)
                    else:
                        sf = act_gate(1, AF.Sigmoid)
                        t1 = wp.tile([128, CH], f32, tag="w")
                        nc.vector.tensor_tensor(out=t1[:], in0=si[:], in1=tg[:],
                                                op=OP.mult)
                        t2 = wp.tile([128, CH], f32, tag="w")
                        nc.vector.tensor_tensor(out=t2[:], in0=sf[:], in1=cst[:, ck],
                                                op=OP.mult)
                        nc.vector.tensor_tensor(out=cst[:, ck], in0=t1[:], in1=t2[:],
                                                op=OP.add)
                    tch = wp.tile([128, CH], f32, tag="w")
                    nc.scalar.activation(out=tch[:], in_=cst[:, ck], func=AF.Tanh)
                    nc.vector.tensor_tensor(out=hst[:, ck].bitcast(f32r),
                                            in0=so[:], in1=tch[:], op=OP.mult)

            # ---- phase 4 (before attention to batch ACT table sets): GRUs ----
            def gru_cell(out_t, x_t, h_t, w_ih, w_hh, bsum, bi, bh):
                def mm2(g):
                    ps = pg.tile([128, EL], f32, tag="g")
                    nc.tensor.matmul(out=ps[:], lhsT=r(w_ih[:, g * 128:(g + 1) * 128]),
                                     rhs=r(x_t[:]), start=True, stop=False)
                    nc.tensor.matmul(out=ps[:], lhsT=r(w_hh[:, g * 128:(g + 1) * 128]),
                                     rhs=r(h_t[:]), start=False, stop=True)
                    return ps

                rr = wp.tile([128, EL], f32, tag="w")
                nc.scalar.activation(out=rr[:], in_=mm2(0)[:], func=AF.Sigmoid,
                                     bias=bsum[:, 0:1])
                zz = wp.tile([128, EL], f32, tag="w")
                nc.scalar.activation(out=zz[:], in_=mm2(1)[:], func=AF.Sigmoid,
                                     bias=bsum[:, 1:2])
                gin = pg.tile([128, EL], f32, tag="g")
                nc.tensor.matmul(out=gin[:], lhsT=r(w_ih[:, 256:384]), rhs=r(x_t[:]),
                                 start=True, stop=True)
                ghn = pg.tile([128, EL], f32, tag="g")
                nc.tensor.matmul(out=ghn[:], lhsT=r(w_hh[:, 256:384]), rhs=r(h_t[:]),
                                 start=True, stop=True)
                hn = wp.tile([128, EL], f32, tag="w")
                nc.scalar.activation(out=hn[:], in_=ghn[:], func=AF.Identity,
                                     bias=bh[:, 2:3])
                rhn = wp.tile([128, EL], f32, tag="w")
                nc.vector.tensor_tensor(out=rhn[:], in0=rr[:], in1=hn[:], op=OP.mult)
                npre = wp.tile([128, EL], f32, tag="w")
                nc.vector.tensor_tensor(out=npre[:], in0=rhn[:], in1=gin[:], op=OP.add)
                nn_ = wp.tile([128, EL], f32, tag="w")
                nc.scalar.activation(out=nn_[:], in_=npre[:], func=AF.Tanh,
                                     bias=bi[:, 2:3])
                d = wp.tile([128, EL], f32, tag="w")
                nc.vector.tensor_tensor(out=d[:], in0=h_t[:], in1=nn_[:], op=OP.subtract)
                zd = wp.tile([128, EL], f32, tag="w")
                nc.vector.tensor_tensor(out=zd[:], in0=zz[:], in1=d[:], op=OP.mult)
                nc.vector.tensor_tensor(out=out_t[:].bitcast(f32r), in0=nn_[:],
                                        in1=zd[:], op=OP.add)

            gru_cell(unew, ifm, ufm, guwi, guwh, gubs, gubi, gubh)
            gru_cell(inew, ufm, ifm, giwi, giwh, gibs, gibi, gibh)

            # ---- phase 3: attention over paths ----
            for j in range(NT):
                ps = pt.tile([128, 128], f32, tag="ptr")
                nc.tensor.transpose(out=ps[:], in_=hst[:, j * 128:(j + 1) * 128],
                                    identity=ident[:])
                nc.vector.tensor_copy(out=hrm[:, j * 128:(j + 1) * 128], in_=ps[:])
            nc.vector.tensor_reduce(
                out=scores[:], in_=hrm[:].rearrange("p (j h) -> p j h", h=128),
                op=OP.max, axis=mybir.AxisListType.X,
            )
            nc.scalar.activation(out=es[:], in_=scores[:], func=AF.Exp)
            for j in range(NT):
                zb = pt.tile([128, 1], f32, tag="ptr")
                nc.tensor.matmul(out=zb[:], lhsT=poolb[:], rhs=es[:, j:j + 1],
                                 start=True, stop=True)
                rzb = gp.tile([128, 1], f32, tag="rzb")
                nc.vector.reciprocal(out=rzb[:], in_=zb[:])
                attp = gp.tile([128, 16], f32, tag="attp")
                nc.vector.tensor_scalar(out=attp[:], in0=poole[:],
                                        scalar1=es[:, j:j + 1], scalar2=rzb[:],
                                        op0=OP.mult, op1=OP.mult)
                pe = pt.tile([128, 16], f32, tag="ptr")
                nc.tensor.matmul(out=pe[:], lhsT=hrm[:, j * 128:(j + 1) * 128],
                                 rhs=attp[:], start=True, stop=True)
                nc.vector.tensor_copy(out=pefm[:, j * 16:(j + 1) * 16].bitcast(f32r),
                                      in_=pe[:])

            # ---- phase 5: fused linears, dot, softplus ----
            def fused(out_t, w0, w1, xin, bias):
                ps = pg.tile([128, EL], f32, tag="g")
                nc.tensor.matmul(out=ps[:], lhsT=r(w0[:]), rhs=r(xin[:]),
                                 start=True, stop=False)
                nc.tensor.matmul(out=ps[:], lhsT=r(w1[:]), rhs=r(pefm[:]),
                                 start=False, stop=True)
                nc.scalar.activation(out=out_t[:], in_=ps[:], func=AF.Identity, bias=bias[:])

            ufu = wp.tile([128, EL], f32, tag="w")
            fused(ufu, ulw0, ulw1, unew, ulb)
            ifu = wp.tile([128, EL], f32, tag="w")
            fused(ifu, ilw0, ilw1, inew, ilb)
            prod = wp.tile([128, EL], f32, tag="w")
            nc.vector.tensor_tensor(out=prod[:].bitcast(f32r), in0=ufu[:],
                                    in1=ifu[:], op=OP.mult)
            dot = pt.tile([1, EL], f32, tag="ptr")
            nc.tensor.matmul(out=dot[:], lhsT=r(ones[:]), rhs=r(prod[:]),
                             start=True, stop=True)
            # softplus(x) = relu(x) + log1p(exp(-|x|))
            ax = gp.tile([1, EL], f32, tag="sp")
            nc.scalar.activation(out=ax[:], in_=dot[:], func=AF.Abs)
            ena = gp.tile([1, EL], f32, tag="sp")
            nc.scalar.activation(out=ena[:], in_=ax[:], func=AF.Exp, scale=-1.0)
            l1p = gp.tile([1, EL], f32, tag="sp")
            nc.scalar.activation(out=l1p[:], in_=ena[:], func=AF.Ln, bias=1.0)
            rx = gp.tile([1, EL], f32, tag="sp")
            nc.scalar.activation(out=rx[:], in_=dot[:], func=AF.Relu)
            nc.vector.tensor_tensor(out=compsb[:], in0=rx[:], in1=l1p[:], op=OP.add)
            nc.sync.dma_start(OUT[:], compsb[:])

    nc.compile()
    return nc


def _prep_shared(inputs):
    """Host-side weight prep (replicated across cores)."""
    f = np.float32

    def a(x):
        return np.ascontiguousarray(np.asarray(x, dtype=f))

    shared = {
        "user_emb": a(inputs["user_emb"]),
        "item_emb": a(inputs["item_emb"]),
        "wih_t": a(np.asarray(inputs["lstm_Wih"]).T),
        "whh_t": a(np.asarray(inputs["lstm_Whh"]).T),
        "lstm_b": a((np.asarray(inputs["lstm_bih"]) + np.asarray(inputs["lstm_bhh"]))
                    .reshape(4, 128).T),
        "guw_ih_t": a(np.asarray(inputs["gru_u_Wih"]).T),
        "guw_hh_t": a(np.asarray(inputs["gru_u_Whh"]).T),
        "giw_ih_t": a(np.asarray(inputs["gru_i_Wih"]).T),
        "giw_hh_t": a(np.asarray(inputs["gru_i_Whh"]).T),
        "gu_bsum": a((np.asarray(inputs["gru_u_bih"]) + np.asarray(inputs["gru_u_bhh"]))
                     .reshape(3, 128).T),
        "gu_bi": a(np.asarray(inputs["gru_u_bih"]).reshape(3, 128).T),
        "gu_bh": a(np.asarray(inputs["gru_u_bhh"]).reshape(3, 128).T),
        "gi_bsum": a((np.asarray(inputs["gru_i_bih"]) + np.asarray(inputs["gru_i_bhh"]))
                     .reshape(3, 128).T),
        "gi_bi": a(np.asarray(inputs["gru_i_bih"]).reshape(3, 128).T),
        "gi_bh": a(np.asarray(inputs["gru_i_bhh"]).reshape(3, 128).T),
        "ulw_t": a(np.asarray(inputs["u_lin_W"]).T),
        "ilw_t": a(np.asarray(inputs["i_lin_W"]).T),
        "ulb": a(np.asarray(inputs["u_lin_b"]).reshape(128, 1)),
        "ilb": a(np.asarray(inputs["i_lin_b"]).reshape(128, 1)),
        "ident": np.eye(128, dtype=f),
        "ones": np.ones((128, 1), f),
    }
    pool_e = np.zeros((128, 16), f)
    pool_e[np.arange(128), np.arange(128) // 8] = 1.0
    shared["pool_e"] = pool_e
    pool_blk = np.zeros((128, 128), f)
    for n in range(128):
        pool_blk[n, (n // 8) * 8:(n // 8) * 8 + 8] = 1.0
    shared["pool_blk"] = pool_blk
    return shared


def _prep_gidx(inputs, c):
    sl = slice(c * EL, (c + 1) * EL)
    pi = np.asarray(inputs["path_idx"][sl]).reshape(N, 4).astype(np.int32)
    cols = [pi[:, t].reshape(NT, 128).T for t in range(4)]
    eu = np.asarray(inputs["event_user"][sl]).reshape(ET, 128).T.astype(np.int32)
    ei = np.asarray(inputs["event_item"][sl]).reshape(ET, 128).T.astype(np.int32)
    return np.ascontiguousarray(
        np.concatenate(cols + [eu, ei], axis=1).astype(np.int32))


def build_in_maps(inputs):
    shared = _prep_shared(inputs)
    return [dict(shared, gidx=_prep_gidx(inputs, c)) for c in range(M)]


def get_program():
    if "nc" not in _CACHE:
        _CACHE["nc"] = _build_program()
    return _CACHE["nc"]


def kernel(**inputs) -> np.ndarray:
    global LAST_RESULT
    from concourse.bass_utils import run_bass_kernel_spmd

    nc = get_program()
    in_maps = build_in_maps(inputs)
    res = run_bass_kernel_spmd(nc, in_maps, list(range(M)))
    LAST_RESULT = res
    out = np.concatenate([res.results[c]["comp"][0] for c in range(M)])
    return out.astype(np.float32)
